# revision 6
# baseline (speedup 1.0000x reference)
"""Distributed Trainium2 kernel for nn_AFMALoss (8 NeuronCores, data-parallel over batch).

Math (per batch b, channel c):
    y_gt    = onehot(target)                          (C,H,W)
    u_gt    = unfold(y_gt, 16)                        (C, 256, 4096)
    u_conv  = unfold(avgpool4x4(y_gt), 16)            (C, 256, 256)
    G       = u_gt^T @ u_conv / 256                   (C, 4096, 256)
    loss    = mean((attentions - G)^2)

Device strategy per core (1 batch each):
  - target is host-permuted to (k, l) "unfold" layout (k = within-patch pixel in
    kappa-order, l = patch index in sigma-order), bf16.  One-hot per channel on
    VectorE (exact in bf16).
  - u_conv is derived on-device: a tiny matmul pools each 4x4 pixel block of
    every patch (BmatV) into v, v bounces through DRAM to land in (k, m)
    layout.  All values are exact in bf16 (multiples of 2^-12).
  - attentions are host-reordered to the sigma row order so each 128-row chunk
    (x 4 channels) is one contiguous 512 KB DMA.
  - G tiles are computed by TensorE (bf16, exact, fp32 PSUM accumulate), then
    VectorE computes D = A - G (exact f32->bf16), ScalarE computes sum(D^2)
    via Square+accum.  Per-core partials are summed on host (mean reduction).
"""

import sys

sys.path.insert(0, "/opt/trn_rl_repo")

import numpy as np
import ml_dtypes

import concourse.bass as bass
import concourse.bacc as bacc
import concourse.mybir as mybir
import concourse.tile as tile
from concourse.bass_utils import run_bass_kernel_spmd

BF16 = ml_dtypes.bfloat16

B, C, H, W = 8, 4, 1024, 1024
P = 16                      # patch
KK = P * P                  # 256 within-patch pixels
L = (H // P) * (W // P)     # 4096 patches
L2 = 256                    # pooled patches
NCHUNK = 32                 # l-chunks of 128
NTOT = float(B * C * L * L2)

_NC_CACHE = {}

# sigma ordering of patches: l = my*256 + dy*64 + mx*4 + dx ; sigma = (dy,dx,my,mx)
_LNAT = np.arange(L).reshape(16, 4, 16, 4)
SIG_OF = np.ascontiguousarray(_LNAT.transpose(1, 3, 0, 2).reshape(L))


def _build_nc():
    nc = bacc.Bacc(None, target_bir_lowering=False)
    f32 = mybir.dt.float32
    bf16 = mybir.dt.bfloat16

    tperm = nc.declare_dram_parameter("tperm", [KK, L], bf16, isOutput=False)
    # att host-reordered: [chunk, partition(=sigma row), c*m]
    att = nc.declare_dram_parameter("att", [NCHUNK, 128, C * L2], f32, isOutput=False)
    bmat = nc.declare_dram_parameter("bmat", [KK, 16], bf16, isOutput=False)
    out = nc.declare_dram_parameter("out", [1, 1], f32, isOutput=True)

    with tile.TileContext(nc) as tc:
        with (
            tc.tile_pool(name="persist", bufs=1) as pp,
            tc.tile_pool(name="awork", bufs=10) as ap_,
            tc.tile_pool(name="dwork", bufs=3) as dp,
            tc.tile_pool(name="dram", bufs=1, space="DRAM") as dr,
            tc.tile_pool(name="psum_d", bufs=3, space="PSUM") as psd,
            tc.tile_pool(name="psum_v", bufs=2, space="PSUM") as psv,
        ):
            # ---- persistent tiles ----
            tp_sb = [pp.tile([128, L], bf16, name=f"tp{kc}", tag=f"tp{kc}") for kc in range(2)]
            bm_sb = [pp.tile([128, 16], bf16, name=f"bm{kc}", tag=f"bm{kc}") for kc in range(2)]
            ugt = [
                [pp.tile([128, L], bf16, name=f"ugt{c}_{kc}", tag=f"ugt{c}_{kc}") for kc in range(2)]
                for c in range(C)
            ]
            vsb = [pp.tile([16, L], bf16, name=f"v{c}", tag=f"v{c}") for c in range(C)]
            ucv = [
                [pp.tile([128, L2], bf16, name=f"uc{c}_{kc}", tag=f"uc{c}_{kc}") for kc in range(2)]
                for c in range(C)
            ]
            vdram = dr.tile([C, 16, L], bf16, name="vdram", tag="vdram")
            acc = pp.tile([128, NCHUNK], f32, name="acc", tag="acc")
            acc1 = pp.tile([128, 1], f32, name="acc1", tag="acc1")
            ones = pp.tile([128, 1], f32, name="ones", tag="ones")
            out_sb = pp.tile([1, 1], f32, name="outsb", tag="outsb")

            # ---- load tperm + bmat ----
            for kc in range(2):
                nc.sync.dma_start(tp_sb[kc][:], tperm[kc * 128:(kc + 1) * 128, :])
                nc.sync.dma_start(bm_sb[kc][:], bmat[kc * 128:(kc + 1) * 128, :])

            # ---- one-hot (VectorE, bf16 4x mode) ----
            for c in range(C):
                for kc in range(2):
                    nc.vector.tensor_scalar(
                        ugt[c][kc][:], tp_sb[kc][:], float(c), None,
                        mybir.AluOpType.is_equal,
                    )

            # ---- v = pooled per-block sums (TensorE) + copy out (ScalarE),
            #      then bounce each half through DRAM into (k, m) layout ----
            for c in range(C):
                for half in range(2):
                    for nt in range(half * 4, half * 4 + 4):
                        vt = psv.tile([16, 512], mybir.dt.float32, name="vps", tag="vps")
                        for kc in range(2):
                            nc.tensor.matmul(
                                vt[:],
                                bm_sb[kc][:],
                                ugt[c][kc][:, nt * 512:(nt + 1) * 512],
                                start=(kc == 0),
                                stop=(kc == 1),
                            )
                        nc.scalar.copy(vsb[c][:, nt * 512:(nt + 1) * 512], vt[:])
                    nc.sync.dma_start(
                        vdram[c][:, half * 2048:(half + 1) * 2048],
                        vsb[c][:, half * 2048:(half + 1) * 2048],
                    )
                    # ucv[c][half][g*16+j, m] = v[j, (half*8+g)*256 + m]
                    src = vdram[c].rearrange("j (g m) -> g j m", g=16)[half * 8:(half + 1) * 8]
                    nc.sync.dma_start(ucv[c][half][:], src)

            # ---- main loop over 32 l-chunks ----
            for q in range(NCHUNK):
                at = ap_.tile([128, C * L2], mybir.dt.float32, name="at", tag="at")
                nc.sync.dma_start(at[:], att[q])
                dps = psd.tile([128, C * L2], mybir.dt.float32, name="dps", tag="dps")
                for c in range(C):
                    for kc in range(2):
                        nc.tensor.matmul(
                            dps[:, c * L2:(c + 1) * L2],
                            ugt[c][kc][:, q * 128:(q + 1) * 128],
                            ucv[c][kc][:],
                            start=(kc == 0),
                            stop=(kc == 1),
                        )
                dsb = dp.tile([128, C * L2], bf16, name="dsb", tag="dsb")
                nc.vector.tensor_tensor(
                    dsb[:], at[:], dps[:], op=mybir.AluOpType.subtract
                )
                sq = dp.tile([128, C * L2], bf16, name="sq", tag="sq")
                nc.scalar.activation(
                    sq[:], dsb[:], mybir.ActivationFunctionType.Square,
                    accum_out=acc[:, q:q + 1],
                )

            # ---- final reduce ----
            nc.vector.memset(ones[:], 1.0)
            nc.vector.reduce_sum(acc1[:], acc[:], axis=mybir.AxisListType.X)
            tot = psv.tile([1, 1], mybir.dt.float32, name="tot", tag="vps")
            nc.tensor.matmul(tot[:], acc1[:], ones[:], start=True, stop=True)
            nc.vector.tensor_scalar_mul(out_sb[:], tot[:], 1.0 / NTOT)
            nc.sync.dma_start(out[:], out_sb[:])

    nc.finalize()
    return nc


def _host_prep(target_b):
    """target (1024,1024) int -> (256, 4096) bf16 in kappa x sigma layout."""
    t8 = np.asarray(target_b).reshape(16, 4, 4, 4, 16, 4, 4, 4)
    # axes: (my, dy, gy, k4y, mx, dx, gx, k4x)
    tp = t8.transpose(2, 6, 3, 7, 1, 5, 0, 4).reshape(KK, L)
    return np.ascontiguousarray(tp).astype(BF16)


def _host_att(att_b):
    """(C, L, L2) f32 -> (NCHUNK, 128, C*L2) with rows in sigma order."""
    a = att_b[:, SIG_OF, :]                    # (C, L, L2) rows sigma-ordered
    a = a.transpose(1, 0, 2)                   # (L, C, L2)
    return np.ascontiguousarray(a).reshape(NCHUNK, 128, C * L2)


def _bmatv():
    return np.ascontiguousarray(
        (np.kron(np.eye(16), np.ones((16, 1))) * 2.0 ** -12).astype(BF16)
    )


def get_nc():
    if "nc" not in _NC_CACHE:
        _NC_CACHE["nc"] = _build_nc()
    return _NC_CACHE["nc"]


def make_in_maps(target, attentions):
    bm = _bmatv()
    att = np.asarray(attentions, dtype=np.float32)
    return [
        {
            "tperm": _host_prep(target[b]),
            "att": _host_att(att[b]),
            "bmat": bm,
        }
        for b in range(B)
    ]


def kernel(pred=None, target=None, attentions=None, **kw):
    nc = get_nc()
    in_maps = make_in_maps(target, attentions)
    res = run_bass_kernel_spmd(nc, in_maps, list(range(B)))
    loss = sum(float(r["out"][0, 0]) for r in res.results)
    return np.float32(loss)


# revision 7
# speedup vs baseline: 1.0506x; 1.0506x over previous
"""Distributed Trainium2 kernel for nn_AFMALoss (8 NeuronCores, data-parallel over batch).

Math (per batch b, channel c):
    y_gt    = onehot(target)                          (C,H,W)
    u_gt    = unfold(y_gt, 16)                        (C, 256, 4096)
    u_conv  = unfold(avgpool4x4(y_gt), 16)            (C, 256, 256)
    G       = u_gt^T @ u_conv / 256                   (C, 4096, 256)
    loss    = mean((attentions - G)^2)

Device strategy per core (1 batch each):
  - target is host-permuted to (k, l) "unfold" layout (k = within-patch pixel in
    kappa-order, l = patch index in sigma-order), bf16.  One-hot per channel on
    VectorE (exact in bf16).
  - u_conv is derived on-device: a tiny matmul pools each 4x4 pixel block of
    every patch (BmatV) into v, v bounces through DRAM to land in (k, m)
    layout.  All values are exact in bf16 (multiples of 2^-12).
  - attentions are host-reordered to the sigma row order so each 128-row chunk
    (x 4 channels) is one contiguous 512 KB DMA.
  - G tiles are computed by TensorE (bf16, exact, fp32 PSUM accumulate), then
    VectorE computes D = A - G (exact f32->bf16), ScalarE computes sum(D^2)
    via Square+accum.  Per-core partials are summed on host (mean reduction).
"""

import sys

sys.path.insert(0, "/opt/trn_rl_repo")

import numpy as np
import ml_dtypes

import concourse.bass as bass
import concourse.bacc as bacc
import concourse.mybir as mybir
import concourse.tile as tile
from concourse.tile import add_dep_helper
from concourse.bass_utils import run_bass_kernel_spmd

BF16 = ml_dtypes.bfloat16

B, C, H, W = 8, 4, 1024, 1024
P = 16                      # patch
KK = P * P                  # 256 within-patch pixels
L = (H // P) * (W // P)     # 4096 patches
L2 = 256                    # pooled patches
NCHUNK = 32                 # l-chunks of 128
NTOT = float(B * C * L * L2)

_NC_CACHE = {}

# sigma ordering of patches: l = my*256 + dy*64 + mx*4 + dx ; sigma = (dy,dx,my,mx)
_LNAT = np.arange(L).reshape(16, 4, 16, 4)
SIG_OF = np.ascontiguousarray(_LNAT.transpose(1, 3, 0, 2).reshape(L))


def _build_nc():
    nc = bacc.Bacc(None, target_bir_lowering=False)
    f32 = mybir.dt.float32
    bf16 = mybir.dt.bfloat16

    tperm = nc.declare_dram_parameter("tperm", [KK, L], bf16, isOutput=False)
    # att host-reordered: [chunk, partition(=sigma row), c*m]
    att = nc.declare_dram_parameter("att", [NCHUNK, 128, C * L2], f32, isOutput=False)
    bmat = nc.declare_dram_parameter("bmat", [KK, 16], bf16, isOutput=False)
    out = nc.declare_dram_parameter("out", [1, 1], f32, isOutput=True)

    with tile.TileContext(nc) as tc:
        with (
            tc.tile_pool(name="persist", bufs=1) as pp,
            tc.tile_pool(name="awork", bufs=10) as ap_,
            tc.tile_pool(name="dwork", bufs=3) as dp,
            tc.tile_pool(name="dram", bufs=1, space="DRAM") as dr,
            tc.tile_pool(name="psum_d", bufs=3, space="PSUM") as psd,
            tc.tile_pool(name="psum_v", bufs=2, space="PSUM") as psv,
        ):
            # ---- persistent tiles ----
            tp_sb = [pp.tile([128, L], bf16, name=f"tp{kc}", tag=f"tp{kc}") for kc in range(2)]
            bm_sb = [pp.tile([128, 16], bf16, name=f"bm{kc}", tag=f"bm{kc}") for kc in range(2)]
            ugt = [
                [pp.tile([128, L], bf16, name=f"ugt{c}_{kc}", tag=f"ugt{c}_{kc}") for kc in range(2)]
                for c in range(C)
            ]
            vsb = [pp.tile([16, L], bf16, name=f"v{c}", tag=f"v{c}") for c in range(C)]
            ucv = [
                [pp.tile([128, L2], bf16, name=f"uc{c}_{kc}", tag=f"uc{c}_{kc}") for kc in range(2)]
                for c in range(C)
            ]
            vdram = dr.tile([C, 16, L], bf16, name="vdram", tag="vdram")
            acc = pp.tile([128, NCHUNK], f32, name="acc", tag="acc")
            acc1 = pp.tile([128, 1], f32, name="acc1", tag="acc1")
            ones = pp.tile([128, 1], f32, name="ones", tag="ones")
            out_sb = pp.tile([1, 1], f32, name="outsb", tag="outsb")

            # ---- load tperm + bmat (must win the DMA bandwidth race) ----
            prio_dmas = []
            for kc in range(2):
                prio_dmas.append(nc.sync.dma_start(tp_sb[kc][:], tperm[kc * 128:(kc + 1) * 128, :]))
                prio_dmas.append(nc.sync.dma_start(bm_sb[kc][:], bmat[kc * 128:(kc + 1) * 128, :]))

            # ---- one-hot (VectorE, bf16 4x mode) ----
            for c in range(C):
                for kc in range(2):
                    nc.vector.tensor_scalar(
                        ugt[c][kc][:], tp_sb[kc][:], float(c), None,
                        mybir.AluOpType.is_equal,
                    )

            # ---- v = pooled per-block sums (TensorE) + copy out (ScalarE),
            #      then bounce each half through DRAM into (k, m) layout ----
            for c in range(C):
                for half in range(2):
                    for nt in range(half * 4, half * 4 + 4):
                        vt = psv.tile([16, 512], mybir.dt.float32, name="vps", tag="vps")
                        for kc in range(2):
                            nc.tensor.matmul(
                                vt[:],
                                bm_sb[kc][:],
                                ugt[c][kc][:, nt * 512:(nt + 1) * 512],
                                start=(kc == 0),
                                stop=(kc == 1),
                            )
                        nc.scalar.copy(vsb[c][:, nt * 512:(nt + 1) * 512], vt[:])
                    nc.sync.dma_start(
                        vdram[c][:, half * 2048:(half + 1) * 2048],
                        vsb[c][:, half * 2048:(half + 1) * 2048],
                    )
                    # ucv[c][half][g*16+j, m] = v[j, (half*8+g)*256 + m]
                    src = vdram[c].rearrange("j (g m) -> g j m", g=16)[half * 8:(half + 1) * 8]
                    nc.sync.dma_start(ucv[c][half][:], src)

            # ---- main loop over 32 l-chunks ----
            for q in range(NCHUNK):
                at = ap_.tile([128, C * L2], mybir.dt.float32, name="at", tag="at")
                atd = nc.sync.dma_start(at[:], att[q])
                if q < 10:
                    for pd in prio_dmas[:2]:
                        add_dep_helper(atd.ins, pd.ins, True, "tperm first")
                dps = psd.tile([128, C * L2], mybir.dt.float32, name="dps", tag="dps")
                for c in range(C):
                    for kc in range(2):
                        nc.tensor.matmul(
                            dps[:, c * L2:(c + 1) * L2],
                            ugt[c][kc][:, q * 128:(q + 1) * 128],
                            ucv[c][kc][:],
                            start=(kc == 0),
                            stop=(kc == 1),
                        )
                dsb = dp.tile([128, C * L2], bf16, name="dsb", tag="dsb")
                nc.vector.tensor_tensor(
                    dsb[:], at[:], dps[:], op=mybir.AluOpType.subtract
                )
                sq = dp.tile([128, C * L2], bf16, name="sq", tag="sq")
                nc.scalar.activation(
                    sq[:], dsb[:], mybir.ActivationFunctionType.Square,
                    accum_out=acc[:, q:q + 1],
                )

            # ---- final reduce ----
            nc.vector.memset(ones[:], 1.0)
            nc.vector.reduce_sum(acc1[:], acc[:], axis=mybir.AxisListType.X)
            tot = psv.tile([1, 1], mybir.dt.float32, name="tot", tag="vps")
            nc.tensor.matmul(tot[:], acc1[:], ones[:], start=True, stop=True)
            nc.vector.tensor_scalar_mul(out_sb[:], tot[:], 1.0 / NTOT)
            nc.sync.dma_start(out[:], out_sb[:])

    nc.finalize()
    return nc


def _host_prep(target_b):
    """target (1024,1024) int -> (256, 4096) bf16 in kappa x sigma layout."""
    t8 = np.asarray(target_b).reshape(16, 4, 4, 4, 16, 4, 4, 4)
    # axes: (my, dy, gy, k4y, mx, dx, gx, k4x)
    tp = t8.transpose(2, 6, 3, 7, 1, 5, 0, 4).reshape(KK, L)
    return np.ascontiguousarray(tp).astype(BF16)


def _host_att(att_b):
    """(C, L, L2) f32 -> (NCHUNK, 128, C*L2) with rows in sigma order."""
    a = att_b[:, SIG_OF, :]                    # (C, L, L2) rows sigma-ordered
    a = a.transpose(1, 0, 2)                   # (L, C, L2)
    return np.ascontiguousarray(a).reshape(NCHUNK, 128, C * L2)


def _bmatv():
    return np.ascontiguousarray(
        (np.kron(np.eye(16), np.ones((16, 1))) * 2.0 ** -12).astype(BF16)
    )


def get_nc():
    if "nc" not in _NC_CACHE:
        _NC_CACHE["nc"] = _build_nc()
    return _NC_CACHE["nc"]


def make_in_maps(target, attentions):
    bm = _bmatv()
    att = np.asarray(attentions, dtype=np.float32)
    return [
        {
            "tperm": _host_prep(target[b]),
            "att": _host_att(att[b]),
            "bmat": bm,
        }
        for b in range(B)
    ]


def kernel(pred=None, target=None, attentions=None, **kw):
    nc = get_nc()
    in_maps = make_in_maps(target, attentions)
    res = run_bass_kernel_spmd(nc, in_maps, list(range(B)))
    loss = sum(float(r["out"][0, 0]) for r in res.results)
    return np.float32(loss)
